# revision 1
# baseline (speedup 1.0000x reference)
"""Trainium2 Bass kernel for nn_BasicBlock (spiking CNN block).

Sharding: data-parallel over batch B across 8 NeuronCores (4 batch x 4
timesteps = 16 images per core); BN batch stats via tiny AllReduce.

Per core:
- conv1: 3x3 taps as TensorEngine matmuls in fp16 hi/lo split arithmetic
  (~fp32 accuracy at bf16 speed): per tap [W1hi;W1hi] x [xhi;xlo] (K=128)
  + W1lo x xhi (K=64). Two images run concurrently via PE column tiling.
- BN stats (sum / sum-of-squares) accumulated during PSUM evacuation
  (ScalarE copy w/ accum_out; DVE square pass), all-reduced across cores.
- PLIF scan in "q-space" (conv-output units): BN scale/bias folded into
  per-channel threshold theta / constants, so no per-element BN apply.
- conv2 consumes exact 0/1 spikes in fp16: per tap [W2hi;W2lo] x [s1;s1]
  (K=128) gives both split terms in one matmul.
- Residual + LIF2 streamed in half-strips; out written via casting DMA.
"""
import sys
sys.path.insert(0, '/opt/trn_rl_repo')

import numpy as np

T, B, C, H, W = 4, 32, 64, 56, 56
NCORES = 8
BL = B // NCORES            # 4 local batch samples
NIMG = T * BL               # 16 images per core
HP = W + 2                  # 58
PP = HP * HP                # 3364 padded pixels
PIX = H * W                 # 3136
NCH = 7                     # conv chunks per image (8 rows each)
CHW = 8 * W                 # 448
NPAIR = 8                   # image pairs per core
EPS = 1e-5
NG = float((T * B) * PIX)   # 401408
QL = 14 * W                 # LIF quarter-strip length (784)
NQ = 4

_prog_cache = {}
DBG = False
NO_CC = False
PHASES = 3
TRACE = False
LAST_RES = None
LAST_NAMES = None
LAST_EXEC_NS = None


def _build(alpha1, alpha2):
    import concourse.mybir as mybir
    import concourse.tile as tile
    from concourse import bacc

    F32 = mybir.dt.float32
    F16 = mybir.dt.float16
    AO = mybir.AluOpType
    AF = mybir.ActivationFunctionType
    AX = mybir.AxisListType

    nc = bacc.Bacc(None, target_bir_lowering=False)
    names = {}

    with tile.TileContext(nc) as tc:
        with tc.tile_pool(name="dram", bufs=1, space="DRAM") as dram:
            xta = dram.tile([NIMG, 2, 64, PP], F16, kind="ExternalInput")
            xin = dram.tile([NIMG, 64, PIX], F32, kind="ExternalInput")
            w1a = dram.tile([128, 9 * 64], F16, kind="ExternalInput")
            w1b = dram.tile([128, 9 * 64], F16, kind="ExternalInput")
            w2a = dram.tile([128, 9 * 64], F16, kind="ExternalInput")
            cpar = dram.tile([128, 8], F32, kind="ExternalInput")
            outp = dram.tile([NIMG, 64, PIX], F32, kind="ExternalOutput")
            names.update(xta=xta.name, xin=xin.name, w1a=w1a.name,
                         w1b=w1b.name, w2a=w2a.name, cpar=cpar.name,
                         outp=outp.name)
            if DBG:
                y1d = dram.tile([NPAIR, 128, PIX], F32, kind="ExternalOutput")
                y2d = dram.tile([NPAIR, 128, PIX], F32, kind="ExternalOutput")
                s1d = dram.tile([NPAIR, 128, PIX], F32, kind="ExternalOutput")
                vecd = dram.tile([128, 8], F32, kind="ExternalOutput")
                names.update(y1d=y1d.name, y2d=y2d.name, s1d=s1d.name,
                             vecd=vecd.name)

            with tc.tile_pool(name="dramw", bufs=1, space="DRAM") as dramw, \
                 tc.tile_pool(name="wsb", bufs=1) as wsb, \
                 tc.tile_pool(name="ys", bufs=8) as yspool, \
                 tc.tile_pool(name="plane", bufs=4) as plpool, \
                 tc.tile_pool(name="hfp", bufs=2) as hf, \
                 tc.tile_pool(name="tiny", bufs=40) as tiny, \
                 tc.tile_pool(name="ps", bufs=7, space="PSUM") as ps:

                # ---- static parameter loads
                w1as = wsb.tile([128, 9 * 64], F16, tag="w1a")
                nc.sync.dma_start(w1as[:], w1a[:])
                w1bs = wsb.tile([128, 9 * 64], F16, tag="w1b")
                nc.sync.dma_start(w1bs[:], w1b[:])
                w2as = wsb.tile([128, 9 * 64], F16, tag="w2a")
                nc.sync.dma_start(w2as[:], w2a[:])
                cpars = wsb.tile([128, 8], F32, tag="cpar")
                nc.sync.dma_start(cpars[:], cpar[:])
                sums1 = wsb.tile([128, 56], F32, tag="sums1")
                sums1q = wsb.tile([128, 56], F32, tag="sums1q")
                sums2 = wsb.tile([128, 56], F32, tag="sums2")
                sums2q = wsb.tile([128, 56], F32, tag="sums2q")
                if PHASES < 2:
                    nc.vector.memset(sums2[:], 0.0)
                    nc.vector.memset(sums2q[:], 0.0)

                def conv_img_pair(plA, plB, lhi, llo, dst_strip, sums_t,
                                  sumsq_t, pcol):
                    """One image pair -> 7 chunks in two waves (4+3); taps
                    outer within a wave so consecutive matmuls hit different
                    PSUM banks and weight loads amortize; wave evacuations
                    overlap the next wave's matmuls."""
                    plAr = plA.rearrange("p (r w) -> p r w", w=HP)
                    plBr = plB.rearrange("p (r w) -> p r w", w=HP)
                    for wave in (range(0, 4), range(4, 7)):
                        pts = {}
                        for cth in wave:
                            pts[cth] = ps.tile([128, CHW], F32, tag="ps",
                                               bufs=7, name=f"psum{cth}")
                        last_a = 8 if llo is None else None
                        for a in range(9):
                            di, dj = a // 3, a % 3
                            for cth in wave:
                                r0 = 8 * cth + di
                                for j, plr in enumerate((plAr, plBr)):
                                    rhs = plr[:, r0:r0 + 8, dj:dj + W]
                                    out = pts[cth][64 * j:64 * (j + 1), :] \
                                        .rearrange("p (r w) -> p r w", r=8)
                                    nc.tensor.matmul(
                                        out, lhi[:, a * 64:(a + 1) * 64], rhs,
                                        start=(a == 0), stop=(a == last_a),
                                        tile_position=(0, 64 * j),
                                        skip_group_check=True)
                        if llo is not None:
                            for a in range(9):
                                di, dj = a // 3, a % 3
                                for cth in wave:
                                    r0 = 8 * cth + di
                                    for j, plr in enumerate((plAr, plBr)):
                                        rhs = plr[:, r0:r0 + 8, dj:dj + W]
                                        out = pts[cth][64 * j:64 * (j + 1), :] \
                                            .rearrange("p (r w) -> p r w", r=8)
                                        nc.tensor.matmul(
                                            out, llo[:, a * 64:(a + 1) * 64], rhs,
                                            start=False, stop=(a == 8),
                                            tile_position=(0, 64 * j),
                                            skip_group_check=True)
                        for cth in wave:
                            nc.scalar.activation(
                                dst_strip[:, CHW * cth:CHW * (cth + 1)],
                                pts[cth][:], AF.Copy,
                                accum_out=sums_t[:, pcol * 7 + cth:pcol * 7 + cth + 1])
                            jk = ps.tile([128, CHW], F32, tag="psjk", bufs=1,
                                         name="psjk")
                            sl = dst_strip[:, CHW * cth:CHW * (cth + 1)]
                            nc.vector.scalar_tensor_tensor(
                                jk[:], sl, 1.0, sl, AO.bypass, AO.mult,
                                accum_out=sumsq_t[:, pcol * 7 + cth:pcol * 7 + cth + 1])

                # ================= phase A: conv1 =================
                y1s = []
                for p in range(NPAIR):
                    tt_, bp = p // 2, p % 2
                    iA = tt_ * 4 + bp * 2
                    planes = []
                    for j in range(2):
                        i = iA + j
                        ta = plpool.tile([128, PP], F16, tag="ta")
                        nc.sync.dma_start(ta[0:64, :], xta[i, 0])
                        nc.sync.dma_start(ta[64:128, :], xta[i, 1])
                        planes.append(ta)
                    strip = yspool.tile([128, PIX], F32, tag="ys")
                    y1s.append(strip)
                    conv_img_pair(planes[0], planes[1], w1as, w1bs, strip,
                                  sums1, sums1q, p)
                    if DBG:
                        nc.sync.dma_start(y1d[p], strip[:])

                # ---- stats1 allreduce
                cc1i = dramw.tile([128, 2], F32)
                cc1o = dramw.tile([128, 2], F32, addr_space="Shared")
                acc1 = tiny.tile([128, 2], F32, tag="acc")
                nc.vector.tensor_reduce(acc1[:, 0:1], sums1[:], AX.X, AO.add)
                nc.vector.tensor_reduce(acc1[:, 1:2], sums1q[:], AX.X, AO.add)
                nc.sync.dma_start(cc1i[:], acc1[:])
                if NO_CC:
                    nc.sync.dma_start(cc1o[:], cc1i[:])
                else:
                    nc.gpsimd.collective_compute(
                        "AllReduce", AO.add, ins=[cc1i[:]], outs=[cc1o[:]],
                        replica_groups=[list(range(NCORES))])
                g1 = tiny.tile([128, 2], F32, tag="acc")
                nc.sync.dma_start(g1[:], cc1o[:])

                shuf_mask = [(i + 16) % 32 for i in range(32)]

                def stats_block(g, gamma, beta, rga, rgam, alpha):
                    gr = tiny.tile([128, 2], F32, tag="acc")
                    nc.sync.dma_start(gr[0:64, :], g[64:128, :])
                    nc.sync.dma_start(gr[64:128, :], g[0:64, :])
                    tot = tiny.tile([128, 2], F32, tag="acc")
                    nc.vector.tensor_tensor(tot[:], g[:], gr[:], AO.add)
                    mean = tiny.tile([128, 1], F32, tag="t1")
                    nc.vector.tensor_scalar(mean[:], tot[:, 0:1], 1.0 / NG,
                                            None, AO.mult)
                    msq = tiny.tile([128, 1], F32, tag="t1")
                    nc.vector.tensor_scalar(msq[:], tot[:, 1:2], 1.0 / NG,
                                            None, AO.mult)
                    m2 = tiny.tile([128, 1], F32, tag="t1")
                    nc.vector.scalar_tensor_tensor(m2[:], mean[:], 1.0, mean[:],
                                                   AO.bypass, AO.mult)
                    var = tiny.tile([128, 1], F32, tag="t1")
                    nc.vector.tensor_tensor(var[:], msq[:], m2[:], AO.subtract)
                    epst = tiny.tile([128, 1], F32, tag="t1")
                    nc.vector.memset(epst[:], EPS)
                    std = tiny.tile([128, 1], F32, tag="t1")
                    nc.scalar.activation(std[:], var[:], AF.Sqrt, bias=epst[:])
                    rstd = tiny.tile([128, 1], F32, tag="t1")
                    nc.vector.reciprocal(rstd[:], std[:])
                    sc = tiny.tile([128, 1], F32, tag="t1")
                    nc.vector.tensor_tensor(sc[:], gamma, rstd[:], AO.mult)
                    nmsc = tiny.tile([128, 1], F32, tag="t1")
                    nc.vector.scalar_tensor_tensor(nmsc[:], mean[:], -1.0, sc[:],
                                                   AO.mult, AO.mult)
                    bi = tiny.tile([128, 1], F32, tag="t1")
                    nc.vector.tensor_tensor(bi[:], beta, nmsc[:], AO.add)
                    stdrg = tiny.tile([128, 1], F32, tag="t1")
                    nc.vector.tensor_tensor(stdrg[:], std[:], rga, AO.mult)
                    nbst = tiny.tile([128, 1], F32, tag="t1")
                    nc.vector.scalar_tensor_tensor(nbst[:], bi[:], -alpha,
                                                   stdrg[:], AO.mult, AO.mult)
                    th = tiny.tile([128, 1], F32, tag="t1")
                    nc.vector.tensor_tensor(th[:], stdrg[:], nbst[:], AO.add)
                    bstd = tiny.tile([128, 1], F32, tag="t1")
                    nc.vector.tensor_tensor(bstd[:], bi[:], std[:], AO.mult)
                    gamv = tiny.tile([128, 1], F32, tag="t1")
                    nc.vector.tensor_tensor(gamv[:], bstd[:], rgam, AO.mult)
                    rscv = tiny.tile([128, 1], F32, tag="t1")
                    nc.vector.tensor_tensor(rscv[:], std[:], rgam, AO.mult)
                    gmw = tiny.tile([128, 1], F32, tag="t1")
                    nc.vector.tensor_scalar(gmw[:], gamv[:], 1.0 - alpha, None,
                                            AO.mult)
                    return th, gamv, rscv, gmw

                th1, gm1, _rsc1, gmw1 = stats_block(
                    g1, cpars[:, 0:1], cpars[:, 1:2], cpars[:, 4:5],
                    cpars[:, 6:7], alpha1)
                if DBG:
                    nc.sync.dma_start(vecd[:, 0:1], th1[:])
                    nc.sync.dma_start(vecd[:, 1:2], gm1[:])
                    nc.sync.dma_start(vecd[:, 4:5], acc1[:, 0:1])
                    nc.sync.dma_start(vecd[:, 5:6], acc1[:, 1:2])

                # ============ phase B + C: LIF1 + conv2 ============
                y2s = [None] * NPAIR
                for bp in range(2 if PHASES >= 2 else 0):
                    Pprev = [None] * NQ
                    for t in range(1, 5):
                        p = (t - 1) * 2 + bp
                        s1tq = []
                        for hq in range(NQ):
                            off = QL * hq
                            ysl = y1s[p][:, off:off + QL]
                            if t == 1:
                                qa = ysl
                            else:
                                q = hf.tile([128, QL], F32, tag="q2", bufs=4)
                                nc.gpsimd.tensor_tensor(q[:], ysl,
                                                        Pprev[hq][:], AO.add)
                                qa = q[:]
                            s1t = hf.tile([128, QL], F16, tag="s1t", bufs=4)
                            nc.vector.tensor_scalar(s1t[:], qa, th1[:],
                                                    None, AO.is_ge)
                            s1tq.append(s1t)
                            if DBG:
                                nc.gpsimd.dma_start(
                                    s1d[p, :, off:off + QL], s1t[:])
                            if t < 4:
                                sb = hf.tile([128, QL], F16, tag="sb", bufs=2)
                                nc.vector.tensor_scalar(sb[:], qa, th1[:],
                                                        None, AO.is_lt)
                                wv = hf.tile([128, QL], F32, tag="q2", bufs=4)
                                nc.vector.tensor_scalar(
                                    wv[:], qa, gm1[:], 1.0 - alpha1,
                                    AO.add, AO.mult)
                                Pn = hf.tile([128, QL], F32, tag="pp", bufs=6)
                                nc.vector.tensor_tensor(Pn[:], wv[:], sb[:],
                                                        AO.mult)
                                Pprev[hq] = Pn
                        iA = (t - 1) * 4 + bp * 2
                        tas_pair = []
                        for j in range(2):
                            tas = plpool.tile([128, PP], F16, tag="ta")
                            tasr = tas.rearrange("p (r w) -> p r w", w=HP)
                            nc.gpsimd.memset(tas[:, 0:HP], 0.0)
                            nc.gpsimd.memset(tas[:, PP - HP:PP], 0.0)
                            nc.gpsimd.memset(tasr[:, :, 0:1], 0.0)
                            nc.gpsimd.memset(tasr[:, :, HP - 1:HP], 0.0)
                            for hq in range(NQ):
                                src = s1tq[hq][64 * j:64 * (j + 1), :] \
                                    .rearrange("p (r w) -> p r w", w=W)
                                dsti = tasr[:, 1 + 14 * hq:1 + 14 * (hq + 1),
                                            1:1 + W]
                                nc.sync.dma_start(dsti[0:64], src)
                                nc.sync.dma_start(dsti[64:128], src)
                            tas_pair.append(tas)
                        strip2 = yspool.tile([128, PIX], F32, tag="ys")
                        y2s[p] = strip2
                        conv_img_pair(tas_pair[0], tas_pair[1], w2as, None,
                                      strip2, sums2, sums2q, p)
                        if DBG:
                            nc.sync.dma_start(y2d[p], strip2[:])

                # ---- stats2 allreduce
                cc2i = dramw.tile([128, 2], F32)
                cc2o = dramw.tile([128, 2], F32, addr_space="Shared")
                acc2 = tiny.tile([128, 2], F32, tag="acc")
                nc.vector.tensor_reduce(acc2[:, 0:1], sums2[:], AX.X, AO.add)
                nc.vector.tensor_reduce(acc2[:, 1:2], sums2q[:], AX.X, AO.add)
                nc.sync.dma_start(cc2i[:], acc2[:])
                if NO_CC:
                    nc.sync.dma_start(cc2o[:], cc2i[:])
                else:
                    nc.gpsimd.collective_compute(
                        "AllReduce", AO.add, ins=[cc2i[:]], outs=[cc2o[:]],
                        replica_groups=[list(range(NCORES))])
                g2 = tiny.tile([128, 2], F32, tag="acc")
                nc.sync.dma_start(g2[:], cc2o[:])
                th2, gm2, rsc2, gmw2 = stats_block(
                    g2, cpars[:, 2:3], cpars[:, 3:4], cpars[:, 5:6],
                    cpars[:, 7:8], alpha2)
                if DBG:
                    nc.sync.dma_start(vecd[:, 2:3], th2[:])
                    nc.sync.dma_start(vecd[:, 3:4], gm2[:])

                # ============ phase D: residual + LIF2 ============
                for bp in range(2 if PHASES >= 3 else 0):
                    Pprev2 = [None] * NQ
                    for t in range(1, 5):
                        p = (t - 1) * 2 + bp
                        iA = (t - 1) * 4 + bp * 2
                        for hq in range(NQ):
                            off = QL * hq
                            xs = hf.tile([128, QL], F32, tag="xs", bufs=6)
                            nc.scalar.dma_start(xs[0:64, :],
                                                xin[iA, :, off:off + QL])
                            nc.scalar.dma_start(xs[64:128, :],
                                                xin[iA + 1, :, off:off + QL])
                            xsc = hf.tile([128, QL], F32, tag="xs", bufs=6)
                            nc.scalar.activation(xsc[:], xs[:], AF.Copy,
                                                 scale=rsc2[:])
                            r = hf.tile([128, QL], F32, tag="xs", bufs=6)
                            nc.gpsimd.tensor_tensor(
                                r[:], xsc[:], y2s[p][:, off:off + QL], AO.add)
                            if t == 1:
                                q2v = r[:]
                            else:
                                q2 = hf.tile([128, QL], F32, tag="q2", bufs=4)
                                nc.vector.tensor_tensor(q2[:], r[:],
                                                        Pprev2[hq][:], AO.add)
                                q2v = q2[:]
                            ot = hf.tile([128, QL], F32, tag="ot", bufs=2)
                            nc.vector.tensor_scalar(ot[:], q2v, th2[:],
                                                    None, AO.is_ge)
                            nc.sync.dma_start(outp[iA, :, off:off + QL],
                                              ot[0:64, :])
                            nc.sync.dma_start(outp[iA + 1, :, off:off + QL],
                                              ot[64:128, :])
                            if t < 4:
                                sb2 = hf.tile([128, QL], F16, tag="sb", bufs=2)
                                nc.vector.tensor_scalar(sb2[:], q2v, th2[:],
                                                        None, AO.is_lt)
                                wv2 = hf.tile([128, QL], F32, tag="q2", bufs=4)
                                nc.scalar.activation(wv2[:], q2v, AF.Identity,
                                                     bias=gmw2[:],
                                                     scale=1.0 - alpha2)
                                Pn = hf.tile([128, QL], F32, tag="pp", bufs=6)
                                nc.vector.tensor_tensor(Pn[:], wv2[:],
                                                        sb2[:], AO.mult)
                                Pprev2[hq] = Pn

    nc.compile()
    return nc, names


def _sigmoid(x):
    return 1.0 / (1.0 + np.exp(-float(x)))


def prepare(x, conv1_w, bn1_gamma, bn1_beta, lif1_w, conv2_w, bn2_gamma,
            bn2_beta, lif2_w):
    x = np.ascontiguousarray(np.asarray(x, np.float32))
    conv1_w = np.asarray(conv1_w, np.float32)
    conv2_w = np.asarray(conv2_w, np.float32)

    a1 = _sigmoid(np.asarray(lif1_w).reshape(-1)[0])
    a2 = _sigmoid(np.asarray(lif2_w).reshape(-1)[0])

    key = (round(a1, 12), round(a2, 12))
    if key not in _prog_cache:
        _prog_cache[key] = _build(a1, a2)
    nc, names = _prog_cache[key]

    # fp16 hi/lo split of x, padded planes (encoding only; exact split)
    xh = x.astype(np.float16)
    xl = (x - xh.astype(np.float32)).astype(np.float16)
    xpad = np.zeros((T, B, C, 2, HP, HP), np.float16)
    xpad[:, :, :, 0, 1:57, 1:57] = xh
    xpad[:, :, :, 1, 1:57, 1:57] = xl
    xpad = np.ascontiguousarray(xpad.transpose(0, 1, 3, 2, 4, 5))  # t,b,2,c,hp,hp

    w1h = conv1_w.astype(np.float16)
    w1l = (conv1_w - w1h.astype(np.float32)).astype(np.float16)
    w2h = conv2_w.astype(np.float16)
    w2l = (conv2_w - w2h.astype(np.float32)).astype(np.float16)

    def tapstack(wtop, wbot):
        out = np.zeros((128, 9 * 64), np.float16)
        for a in range(9):
            di, dj = a // 3, a % 3
            out[0:64, a * 64:(a + 1) * 64] = wtop[:, :, di, dj].T
            out[64:128, a * 64:(a + 1) * 64] = wbot[:, :, di, dj].T
        return out

    w1a_np = tapstack(w1h, w1h)
    w1b_np = tapstack(w1l, w1l)
    w2a_np = tapstack(w2h, w2l)

    def dup(v):
        v = np.asarray(v, np.float32).reshape(64)
        return np.concatenate([v, v])

    cpar_np = np.zeros((128, 8), np.float32)
    cpar_np[:, 0] = dup(bn1_gamma)
    cpar_np[:, 1] = dup(bn1_beta)
    cpar_np[:, 2] = dup(bn2_gamma)
    cpar_np[:, 3] = dup(bn2_beta)
    cpar_np[:, 4] = 1.0 / (a1 * dup(bn1_gamma))
    cpar_np[:, 5] = 1.0 / (a2 * dup(bn2_gamma))
    cpar_np[:, 6] = 1.0 / dup(bn1_gamma)
    cpar_np[:, 7] = 1.0 / dup(bn2_gamma)

    in_maps = []
    for k in range(NCORES):
        xta_np = np.ascontiguousarray(
            xpad[:, 4 * k:4 * k + 4].reshape(NIMG, 2, 64, PP))
        xin_np = np.ascontiguousarray(
            x[:, 4 * k:4 * k + 4].reshape(NIMG, 64, PIX))
        in_maps.append({
            names['xta']: xta_np,
            names['xin']: xin_np,
            names['w1a']: w1a_np,
            names['w1b']: w1b_np,
            names['w2a']: w2a_np,
            names['cpar']: cpar_np,
        })

    return nc, names, in_maps


def kernel(**inputs):
    from concourse.bass_utils import run_bass_kernel_spmd
    nc, names, in_maps = prepare(**inputs)
    res = run_bass_kernel_spmd(nc, in_maps, core_ids=list(range(NCORES)))
    global LAST_RES, LAST_NAMES
    LAST_RES, LAST_NAMES = res, names
    out = np.empty((T, B, C, H, W), np.float32)
    for k in range(NCORES):
        o = res.results[k][names['outp']]
        out[:, 4 * k:4 * k + 4] = o.reshape(T, BL, C, H, W)
    return out


if __name__ == "__main__":
    rng = np.random.default_rng(0)
    xs = rng.standard_normal((T, B, C, H, W)).astype(np.float32)
    w1 = (rng.standard_normal((64, 64, 3, 3)) * 0.05).astype(np.float32)
    w2 = (rng.standard_normal((64, 64, 3, 3)) * 0.05).astype(np.float32)
    o = kernel(xs, w1, np.ones(64, np.float32), np.zeros(64, np.float32),
               np.zeros(1, np.float32), w2, np.ones(64, np.float32),
               np.zeros(64, np.float32), np.zeros(1, np.float32))
    print("ran:", o.shape, float(o.mean()))



# revision 2
# speedup vs baseline: 1.8875x; 1.8875x over previous
"""Trainium2 Bass kernel for nn_BasicBlock (spiking CNN block).

Sharding: data-parallel over batch B across 8 NeuronCores (4 batch x 4
timesteps = 16 images per core); BN batch stats via tiny AllReduce.

Per core (v2 — PE-lean rework):
- conv1: per tap, ONE fp16 matmul with block-diagonal weights computes the
  main term Wh@xh for BOTH images of a pair (K=128=[xhA;xhB], M=128), plus
  ONE fp8e4 DoubleRow matmul computing 512*(Wh@xl + Wl@xh) for both images
  (planes = per-image cross encodings [512*xl; xh/8]); the two PSUM tiles
  are combined at evacuation with scalar_tensor_tensor (out = X/512 + M).
- conv2 consumes exact 0/1 spikes: fp8e4 DoubleRow pass1 per tap
  (slots: w20@s + (64*w21)@(s/64)), plus a tap-paired e5m2 pass2 carrying
  the 2^-12-level correction (64*w22)@(s/64). Spike planes are written
  directly by DVE is_ge ops into padded fp8 plane tiles (no DMA).
- BN stats (sum / sum-of-squares) accumulated during PSUM evacuation,
  all-reduced across cores. PLIF scans run in q-space (BN folded into
  per-channel thresholds), state kept in fp32.
"""
import sys
sys.path.insert(0, '/opt/trn_rl_repo')

import numpy as np

T, B, C, H, W = 4, 32, 64, 56, 56
NCORES = 8
BL = B // NCORES            # 4 local batch samples
NIMG = T * BL               # 16 images per core
HP = W + 2                  # 58
PP = HP * HP                # 3364 padded pixels
PIX = H * W                 # 3136
NCH = 7                     # conv chunks per image (8 rows each)
CHW = 8 * W                 # 448
NPAIR = 8                   # image pairs per core
EPS = 1e-5
NG = float((T * B) * PIX)   # 401408
QL = 14 * W                 # LIF quarter-strip length (784)
NQ = 4
XSC = 512.0                 # conv1 cross-stream PSUM scale
# conv2 pass2 tap pairing: 4 pairs + 1 single (single repeats itself with
# zero weights in plane1)
TAP_PAIRS = [((0, 0), (0, 1)), ((0, 2), (1, 0)), ((1, 1), (1, 2)),
             ((2, 0), (2, 1)), ((2, 2), (2, 2))]

_prog_cache = {}
DBG = False
NO_CC = False
PHASES = 3
TRACE = False
LAST_RES = None
LAST_NAMES = None
LAST_EXEC_NS = None


def _build(alpha1, alpha2):
    import concourse.mybir as mybir
    import concourse.tile as tile
    from concourse import bacc
    from concourse.ap import AP as BassAP

    F32 = mybir.dt.float32
    F16 = mybir.dt.float16
    F8 = mybir.dt.float8e4
    F8_5 = mybir.dt.float8e5
    AO = mybir.AluOpType
    AF = mybir.ActivationFunctionType
    AX = mybir.AxisListType
    DR = mybir.MatmulPerfMode.DoubleRow

    nc = bacc.Bacc(None, target_bir_lowering=False)
    names = {}

    with tile.TileContext(nc) as tc:
        with tc.tile_pool(name="dram", bufs=1, space="DRAM") as dram:
            xmain = dram.tile([NPAIR, 128, PP], F16, kind="ExternalInput")
            xcross = dram.tile([NPAIR, 128, 2, PP], F8, kind="ExternalInput")
            xin = dram.tile([NIMG, 64, PIX], F32, kind="ExternalInput")
            w1m = dram.tile([128, 9 * 128], F16, kind="ExternalInput")
            w1x = dram.tile([128, 2, 9 * 128], F8, kind="ExternalInput")
            w2a = dram.tile([128, 2, 9 * 128], F8, kind="ExternalInput")
            w2b = dram.tile([128, 2, 5 * 128], F8_5, kind="ExternalInput")
            cpar = dram.tile([128, 8], F32, kind="ExternalInput")
            outp = dram.tile([NIMG, 64, PIX], F32, kind="ExternalOutput")
            names.update(xmain=xmain.name, xcross=xcross.name, xin=xin.name,
                         w1m=w1m.name, w1x=w1x.name, w2a=w2a.name,
                         w2b=w2b.name, cpar=cpar.name, outp=outp.name)
            if DBG:
                y1d = dram.tile([NPAIR, 128, PIX], F32, kind="ExternalOutput")
                y2d = dram.tile([NPAIR, 128, PIX], F32, kind="ExternalOutput")
                s1d = dram.tile([NPAIR, 128, 2, PP], F8, kind="ExternalOutput")
                vecd = dram.tile([128, 8], F32, kind="ExternalOutput")
                names.update(y1d=y1d.name, y2d=y2d.name, s1d=s1d.name,
                             vecd=vecd.name)

            with tc.tile_pool(name="dramw", bufs=1, space="DRAM") as dramw, \
                 tc.tile_pool(name="wsb", bufs=1) as wsb, \
                 tc.tile_pool(name="ys", bufs=8) as yspool, \
                 tc.tile_pool(name="xpl", bufs=2) as xpl, \
                 tc.tile_pool(name="spl", bufs=1) as splp, \
                 tc.tile_pool(name="hf", bufs=2) as hf, \
                 tc.tile_pool(name="scr", bufs=2) as scr, \
                 tc.tile_pool(name="tiny", bufs=40) as tiny, \
                 tc.tile_pool(name="ps", bufs=8, space="PSUM") as ps:

                # ---- static parameter loads
                w1ms = wsb.tile([128, 9 * 128], F16, tag="w1m")
                nc.sync.dma_start(w1ms[:], w1m[:])
                w1xs = wsb.tile([128, 2, 9 * 128], F8, tag="w1x")
                nc.sync.dma_start(w1xs[:], w1x[:])
                w2as = wsb.tile([128, 2, 9 * 128], F8, tag="w2a")
                nc.sync.dma_start(w2as[:], w2a[:])
                w2bs = wsb.tile([128, 2, 5 * 128], F8_5, tag="w2b")
                nc.sync.dma_start(w2bs[:], w2b[:])
                cpars = wsb.tile([128, 8], F32, tag="cpar")
                nc.sync.dma_start(cpars[:], cpar[:])
                sums1 = wsb.tile([128, 56], F32, tag="sums1")
                sums1q = wsb.tile([128, 56], F32, tag="sums1q")
                sums2 = wsb.tile([128, 56], F32, tag="sums2")
                sums2q = wsb.tile([128, 56], F32, tag="sums2q")

                # ---- persistent conv2 spike planes (2 slots), pad zeroed once
                NSPL = 2
                splanes = []
                for si in range(NSPL):
                    sp = splp.tile([128, 2, PP], F8, tag=f"spl{si}", bufs=1,
                                   name=f"spl{si}")
                    spr = sp.rearrange("p two (h w) -> p two h w", w=HP)
                    nc.vector.memset(spr[:, :, 0, :], 0.0)
                    nc.vector.memset(spr[:, :, HP - 1, :], 0.0)
                    nc.vector.memset(spr[:, :, :, 0], 0.0)
                    nc.vector.memset(spr[:, :, :, HP - 1], 0.0)
                    splanes.append(sp)

                # ================= phase A: conv1 =================
                y1s = []
                for p in range(NPAIR):
                    xm = xpl.tile([128, PP], F16, tag="xm", bufs=2)
                    nc.sync.dma_start(xm[:], xmain[p])
                    xc = xpl.tile([128, 2, PP], F8, tag="xc", bufs=2)
                    nc.sync.dma_start(xc[:], xcross[p])
                    xmr = xm.rearrange("p (h w) -> p h w", w=HP)
                    xcr = xc.rearrange("p two (h w) -> p two h w", w=HP)
                    strip = yspool.tile([128, PIX], F32, tag="ys")
                    y1s.append(strip)
                    for wave in (range(0, 4), range(4, 7)):
                        ptsM = {}
                        ptsX = {}
                        for cth in wave:
                            ptsM[cth] = ps.tile([128, CHW], F32, tag="ps",
                                                bufs=8, name=f"psm{cth}")
                            ptsX[cth] = ps.tile([128, CHW], F32, tag="ps",
                                                bufs=8, name=f"psx{cth}")
                        for a in range(9):
                            di, dj = a // 3, a % 3
                            for cth in wave:
                                r0 = 8 * cth + di
                                outM = ptsM[cth][:] \
                                    .rearrange("p (r w) -> p r w", r=8)
                                nc.tensor.matmul(
                                    outM, w1ms[:, a * 128:(a + 1) * 128],
                                    xmr[:, r0:r0 + 8, dj:dj + W],
                                    start=(a == 0), stop=(a == 8),
                                    skip_group_check=True)
                                outX = ptsX[cth][:] \
                                    .rearrange("p (r w) -> p r w", r=8)
                                nc.tensor.matmul(
                                    outX, w1xs[:, :, a * 128:(a + 1) * 128],
                                    xcr[:, :, r0:r0 + 8, dj:dj + W],
                                    start=(a == 0), stop=(a == 8),
                                    perf_mode=DR, skip_group_check=True)
                        for cth in wave:
                            sl = strip[:, CHW * cth:CHW * (cth + 1)]
                            nc.vector.scalar_tensor_tensor(
                                sl, ptsX[cth][:], 1.0 / XSC, ptsM[cth][:],
                                AO.mult, AO.add,
                                accum_out=sums1[:, p * 7 + cth:p * 7 + cth + 1])
                            sq = scr.tile([128, CHW], F32, tag="sq", bufs=2)
                            nc.scalar.activation(
                                sq[:], sl, AF.Square,
                                accum_out=sums1q[:, p * 7 + cth:p * 7 + cth + 1])
                    if DBG:
                        nc.sync.dma_start(y1d[p], strip[:])

                # ---- stats1 allreduce
                cc1i = dramw.tile([128, 2], F32)
                cc1o = dramw.tile([128, 2], F32, addr_space="Shared")
                acc1 = tiny.tile([128, 2], F32, tag="acc")
                nc.vector.tensor_reduce(acc1[:, 0:1], sums1[:], AX.X, AO.add)
                nc.vector.tensor_reduce(acc1[:, 1:2], sums1q[:], AX.X, AO.add)
                nc.sync.dma_start(cc1i[:], acc1[:])
                if NO_CC:
                    nc.sync.dma_start(cc1o[:], cc1i[:])
                else:
                    nc.gpsimd.collective_compute(
                        "AllReduce", AO.add, ins=[cc1i[:]], outs=[cc1o[:]],
                        replica_groups=[list(range(NCORES))])
                g1 = tiny.tile([128, 2], F32, tag="acc")
                nc.sync.dma_start(g1[:], cc1o[:])

                def stats_block(g, gamma, beta, rga, rgam, alpha):
                    gr = tiny.tile([128, 2], F32, tag="acc")
                    nc.sync.dma_start(gr[0:64, :], g[64:128, :])
                    nc.sync.dma_start(gr[64:128, :], g[0:64, :])
                    tot = tiny.tile([128, 2], F32, tag="acc")
                    nc.vector.tensor_tensor(tot[:], g[:], gr[:], AO.add)
                    mean = tiny.tile([128, 1], F32, tag="t1")
                    nc.vector.tensor_scalar(mean[:], tot[:, 0:1], 1.0 / NG,
                                            None, AO.mult)
                    msq = tiny.tile([128, 1], F32, tag="t1")
                    nc.vector.tensor_scalar(msq[:], tot[:, 1:2], 1.0 / NG,
                                            None, AO.mult)
                    m2 = tiny.tile([128, 1], F32, tag="t1")
                    nc.vector.scalar_tensor_tensor(m2[:], mean[:], 1.0, mean[:],
                                                   AO.bypass, AO.mult)
                    var = tiny.tile([128, 1], F32, tag="t1")
                    nc.vector.tensor_tensor(var[:], msq[:], m2[:], AO.subtract)
                    epst = tiny.tile([128, 1], F32, tag="t1")
                    nc.vector.memset(epst[:], EPS)
                    std = tiny.tile([128, 1], F32, tag="t1")
                    nc.scalar.activation(std[:], var[:], AF.Sqrt, bias=epst[:])
                    rstd = tiny.tile([128, 1], F32, tag="t1")
                    nc.vector.reciprocal(rstd[:], std[:])
                    sc = tiny.tile([128, 1], F32, tag="t1")
                    nc.vector.tensor_tensor(sc[:], gamma, rstd[:], AO.mult)
                    nmsc = tiny.tile([128, 1], F32, tag="t1")
                    nc.vector.scalar_tensor_tensor(nmsc[:], mean[:], -1.0, sc[:],
                                                   AO.mult, AO.mult)
                    bi = tiny.tile([128, 1], F32, tag="t1")
                    nc.vector.tensor_tensor(bi[:], beta, nmsc[:], AO.add)
                    stdrg = tiny.tile([128, 1], F32, tag="t1")
                    nc.vector.tensor_tensor(stdrg[:], std[:], rga, AO.mult)
                    nbst = tiny.tile([128, 1], F32, tag="t1")
                    nc.vector.scalar_tensor_tensor(nbst[:], bi[:], -alpha,
                                                   stdrg[:], AO.mult, AO.mult)
                    th = tiny.tile([128, 1], F32, tag="t1")
                    nc.vector.tensor_tensor(th[:], stdrg[:], nbst[:], AO.add)
                    bstd = tiny.tile([128, 1], F32, tag="t1")
                    nc.vector.tensor_tensor(bstd[:], bi[:], std[:], AO.mult)
                    gamv = tiny.tile([128, 1], F32, tag="t1")
                    nc.vector.tensor_tensor(gamv[:], bstd[:], rgam, AO.mult)
                    rscv = tiny.tile([128, 1], F32, tag="t1")
                    nc.vector.tensor_tensor(rscv[:], std[:], rgam, AO.mult)
                    gmw = tiny.tile([128, 1], F32, tag="t1")
                    nc.vector.tensor_scalar(gmw[:], gamv[:], 1.0 - alpha, None,
                                            AO.mult)
                    return th, gamv, rscv, gmw

                th1, gm1, _rsc1, gmw1 = stats_block(
                    g1, cpars[:, 0:1], cpars[:, 1:2], cpars[:, 4:5],
                    cpars[:, 6:7], alpha1)
                if DBG:
                    nc.sync.dma_start(vecd[:, 0:1], th1[:])
                    nc.sync.dma_start(vecd[:, 1:2], gm1[:])
                    nc.sync.dma_start(vecd[:, 4:5], acc1[:, 0:1])
                    nc.sync.dma_start(vecd[:, 5:6], acc1[:, 1:2])

                # ============ phase B + C: LIF1 + conv2 ============
                y2s = [None] * NPAIR
                Pprev = {0: [None] * NQ, 1: [None] * NQ}
                for t in range(1, 5 if PHASES >= 2 else 1):
                    for bp in range(2):
                        p = (t - 1) * 2 + bp
                        spl = splanes[p % NSPL]
                        splr = spl.rearrange("p two (h w) -> p two h w", w=HP)
                        for hq in range(NQ):
                            off = QL * hq
                            ysl = y1s[p][:, off:off + QL]
                            if t == 1:
                                qa = ysl
                            else:
                                q = hf.tile([128, QL], F32, tag="tmp", bufs=5)
                                nc.gpsimd.tensor_tensor(q[:], ysl,
                                                        Pprev[bp][hq][:], AO.add)
                                qa = q[:]
                            qar = qa.rearrange("p (r w) -> p r w", w=W)
                            rows = slice(1 + 14 * hq, 1 + 14 * (hq + 1))
                            nc.vector.tensor_scalar(
                                splr[:, 0, rows, 1:1 + W], qar, th1[:],
                                None, AO.is_ge)
                            nc.vector.tensor_scalar(
                                splr[:, 1, rows, 1:1 + W], qar, th1[:],
                                1.0 / 64, AO.is_ge, AO.mult)
                            if t < 4:
                                wv = hf.tile([128, QL], F32, tag="tmp", bufs=5)
                                nc.scalar.activation(wv[:], qa, AF.Identity,
                                                     bias=gmw1[:],
                                                     scale=1.0 - alpha1)
                                Pn = hf.tile([128, QL], F32, tag="pp", bufs=8)
                                nc.vector.scalar_tensor_tensor(
                                    Pn[:], qa, th1[:], wv[:], AO.is_lt, AO.mult)
                                Pprev[bp][hq] = Pn
                        if DBG:
                            nc.sync.dma_start(s1d[p], spl[:])

                        # ---- conv2 for pair p
                        strip2 = yspool.tile([128, PIX], F32, tag="ys")
                        y2s[p] = strip2
                        spl_base = spl[:].offset
                        spl_pstride = spl[:].ap[0][0]
                        for wave in (range(0, 4), range(4, 7)):
                            pts = {}
                            for cth in wave:
                                pts[cth] = ps.tile([128, CHW], F32, tag="ps",
                                                   bufs=8, name=f"ps2{cth}")
                            for a in range(9):
                                di, dj = a // 3, a % 3
                                for cth in wave:
                                    r0 = 8 * cth + di
                                    out2 = pts[cth][:] \
                                        .rearrange("p (r w) -> p r w", r=8)
                                    nc.tensor.matmul(
                                        out2, w2as[:, :, a * 128:(a + 1) * 128],
                                        splr[:, :, r0:r0 + 8, dj:dj + W],
                                        start=(a == 0), stop=False,
                                        perf_mode=DR, skip_group_check=True)
                            for im, (ta, tb) in enumerate(TAP_PAIRS):
                                for cth in wave:
                                    offA = (spl_base + PP
                                            + (8 * cth + ta[0]) * HP + ta[1])
                                    dstr = ((tb[0] - ta[0]) * HP
                                            + (tb[1] - ta[1]))
                                    if dstr == 0:
                                        dstr = 1  # single: plane1 weights are 0
                                    rhs2 = BassAP(
                                        spl[:].tensor, offA,
                                        [[spl_pstride, 128], [dstr, 2],
                                         [HP, 8], [1, W]])
                                    out2 = pts[cth][:] \
                                        .rearrange("p (r w) -> p r w", r=8)
                                    nc.tensor.matmul(
                                        out2, w2bs[:, :, im * 128:(im + 1) * 128],
                                        rhs2, start=False, stop=(im == 4),
                                        perf_mode=DR, skip_group_check=True)
                            for cth in wave:
                                sl2 = strip2[:, CHW * cth:CHW * (cth + 1)]
                                nc.scalar.activation(
                                    sl2, pts[cth][:], AF.Copy,
                                    accum_out=sums2[:, p * 7 + cth:p * 7 + cth + 1])
                                sq2 = scr.tile([128, CHW], F32, tag="sq",
                                               bufs=2)
                                nc.vector.scalar_tensor_tensor(
                                    sq2[:], pts[cth][:], 1.0, pts[cth][:],
                                    AO.bypass, AO.mult,
                                    accum_out=sums2q[:, p * 7 + cth:p * 7 + cth + 1])
                        if DBG:
                            nc.sync.dma_start(y2d[p], strip2[:])

                # ---- stats2 allreduce
                cc2i = dramw.tile([128, 2], F32)
                cc2o = dramw.tile([128, 2], F32, addr_space="Shared")
                acc2 = tiny.tile([128, 2], F32, tag="acc")
                nc.vector.tensor_reduce(acc2[:, 0:1], sums2[:], AX.X, AO.add)
                nc.vector.tensor_reduce(acc2[:, 1:2], sums2q[:], AX.X, AO.add)
                nc.sync.dma_start(cc2i[:], acc2[:])
                if NO_CC:
                    nc.sync.dma_start(cc2o[:], cc2i[:])
                else:
                    nc.gpsimd.collective_compute(
                        "AllReduce", AO.add, ins=[cc2i[:]], outs=[cc2o[:]],
                        replica_groups=[list(range(NCORES))])
                g2 = tiny.tile([128, 2], F32, tag="acc")
                nc.sync.dma_start(g2[:], cc2o[:])
                th2, gm2, rsc2, gmw2 = stats_block(
                    g2, cpars[:, 2:3], cpars[:, 3:4], cpars[:, 5:6],
                    cpars[:, 7:8], alpha2)
                if DBG:
                    nc.sync.dma_start(vecd[:, 2:3], th2[:])
                    nc.sync.dma_start(vecd[:, 3:4], gm2[:])

                # ============ phase D: residual + LIF2 ============
                xinr = xin.rearrange("i c x -> (i c) x")
                outr = outp.rearrange("i c x -> (i c) x")
                Pprev2 = {0: [None] * NQ, 1: [None] * NQ}
                for t in range(1, 5 if PHASES >= 3 else 1):
                    for bp in range(2):
                        p = (t - 1) * 2 + bp
                        iA = (t - 1) * 4 + bp * 2
                        for hq in range(NQ):
                            off = QL * hq
                            xs = hf.tile([128, QL], F32, tag="xs", bufs=3)
                            nc.sync.dma_start(
                                xs[:], xinr[64 * iA:64 * (iA + 2),
                                            off:off + QL])
                            xsc = hf.tile([128, QL], F32, tag="tmp", bufs=5)
                            nc.scalar.activation(xsc[:], xs[:], AF.Copy,
                                                 scale=rsc2[:])
                            r = hf.tile([128, QL], F32, tag="tmp", bufs=5)
                            nc.gpsimd.tensor_tensor(
                                r[:], xsc[:], y2s[p][:, off:off + QL], AO.add)
                            if t == 1:
                                q2v = r[:]
                            else:
                                q2 = hf.tile([128, QL], F32, tag="tmp", bufs=5)
                                nc.vector.tensor_tensor(q2[:], r[:],
                                                        Pprev2[bp][hq][:],
                                                        AO.add)
                                q2v = q2[:]
                            ot = hf.tile([128, QL], F32, tag="ot", bufs=2)
                            nc.vector.tensor_scalar(ot[:], q2v, th2[:],
                                                    None, AO.is_ge)
                            nc.sync.dma_start(
                                outr[64 * iA:64 * (iA + 2), off:off + QL],
                                ot[:])
                            if t < 4:
                                wv2 = hf.tile([128, QL], F32, tag="tmp",
                                              bufs=5)
                                nc.scalar.activation(wv2[:], q2v, AF.Identity,
                                                     bias=gmw2[:],
                                                     scale=1.0 - alpha2)
                                Pn2 = hf.tile([128, QL], F32, tag="pp",
                                              bufs=8)
                                nc.vector.scalar_tensor_tensor(
                                    Pn2[:], q2v, th2[:], wv2[:],
                                    AO.is_lt, AO.mult)
                                Pprev2[bp][hq] = Pn2

    nc.compile()
    return nc, names


def _sigmoid(x):
    return 1.0 / (1.0 + np.exp(-float(x)))


def prepare(x, conv1_w, bn1_gamma, bn1_beta, lif1_w, conv2_w, bn2_gamma,
            bn2_beta, lif2_w):
    import ml_dtypes
    E4 = ml_dtypes.float8_e4m3
    E5 = ml_dtypes.float8_e5m2

    x = np.ascontiguousarray(np.asarray(x, np.float32))
    conv1_w = np.asarray(conv1_w, np.float32)
    conv2_w = np.asarray(conv2_w, np.float32)

    a1 = _sigmoid(np.asarray(lif1_w).reshape(-1)[0])
    a2 = _sigmoid(np.asarray(lif2_w).reshape(-1)[0])

    key = (round(a1, 12), round(a2, 12))
    if key not in _prog_cache:
        _prog_cache[key] = _build(a1, a2)
    nc, names = _prog_cache[key]

    # conv1 splits
    xh = x.astype(np.float16)
    xl = x - xh.astype(np.float32)
    w1h = conv1_w.astype(np.float16).astype(np.float32)
    w1l = conv1_w - w1h
    w1h8 = w1h.astype(E4)                     # cross-stream Wh
    w1l8 = (4096.0 * w1l).astype(E4)          # cross-stream 4096*Wl

    def pad_pair(ahi, alo):
        # -> [128, HP, HP] from two [C, H, W] channel images
        out = np.zeros((128, HP, HP), np.float32)
        out[0:64, 1:57, 1:57] = ahi
        out[64:128, 1:57, 1:57] = alo
        return out

    xh_t = xh.astype(np.float32).reshape(T, BL * NCORES, C, H, W)
    xl_t = xl.reshape(T, BL * NCORES, C, H, W)

    # conv2 splits
    w20 = conv2_w.astype(E4)
    w21s = (64.0 * (conv2_w - w20.astype(np.float32))).astype(E4)
    w22s = (64.0 * (conv2_w - w20.astype(np.float32)
                    - w21s.astype(np.float32) / 64.0)).astype(E5)

    def tap_T(warr, a):
        di, dj = a // 3, a % 3
        return warr[:, :, di, dj].T  # [in, out]

    w1m_np = np.zeros((128, 9 * 128), np.float16)
    w1x_np = np.zeros((128, 2, 9 * 128), E4)
    w2a_np = np.zeros((128, 2, 9 * 128), E4)
    for a in range(9):
        w1m_np[0:64, a * 128:a * 128 + 64] = tap_T(w1h, a).astype(np.float16)
        w1m_np[64:128, a * 128 + 64:a * 128 + 128] = \
            tap_T(w1h, a).astype(np.float16)
        # cross lhsT: plane0 -> imgA out cols 0:64, plane1 -> imgB out cols
        w1x_np[0:64, 0, a * 128:a * 128 + 64] = tap_T(
            w1h8.astype(np.float32), a).astype(E4)
        w1x_np[64:128, 0, a * 128:a * 128 + 64] = tap_T(
            w1l8.astype(np.float32), a).astype(E4)
        w1x_np[0:64, 1, a * 128 + 64:a * 128 + 128] = tap_T(
            w1h8.astype(np.float32), a).astype(E4)
        w1x_np[64:128, 1, a * 128 + 64:a * 128 + 128] = tap_T(
            w1l8.astype(np.float32), a).astype(E4)
        # conv2 pass1: plane0 = blockdiag(w20), plane1 = blockdiag(64*w21)
        w2a_np[0:64, 0, a * 128:a * 128 + 64] = tap_T(
            w20.astype(np.float32), a).astype(E4)
        w2a_np[64:128, 0, a * 128 + 64:a * 128 + 128] = tap_T(
            w20.astype(np.float32), a).astype(E4)
        w2a_np[0:64, 1, a * 128:a * 128 + 64] = tap_T(
            w21s.astype(np.float32), a).astype(E4)
        w2a_np[64:128, 1, a * 128 + 64:a * 128 + 128] = tap_T(
            w21s.astype(np.float32), a).astype(E4)

    w2b_np = np.zeros((128, 2, 5 * 128), E5)
    for im, (ta, tb) in enumerate(TAP_PAIRS):
        aA = ta[0] * 3 + ta[1]
        aB = tb[0] * 3 + tb[1]
        wA = tap_T(w22s.astype(np.float32), aA).astype(E5)
        w2b_np[0:64, 0, im * 128:im * 128 + 64] = wA
        w2b_np[64:128, 0, im * 128 + 64:im * 128 + 128] = wA
        if (ta != tb):
            wB = tap_T(w22s.astype(np.float32), aB).astype(E5)
            w2b_np[0:64, 1, im * 128:im * 128 + 64] = wB
            w2b_np[64:128, 1, im * 128 + 64:im * 128 + 128] = wB

    def dup(v):
        v = np.asarray(v, np.float32).reshape(64)
        return np.concatenate([v, v])

    cpar_np = np.zeros((128, 8), np.float32)
    cpar_np[:, 0] = dup(bn1_gamma)
    cpar_np[:, 1] = dup(bn1_beta)
    cpar_np[:, 2] = dup(bn2_gamma)
    cpar_np[:, 3] = dup(bn2_beta)
    cpar_np[:, 4] = 1.0 / (a1 * dup(bn1_gamma))
    cpar_np[:, 5] = 1.0 / (a2 * dup(bn2_gamma))
    cpar_np[:, 6] = 1.0 / dup(bn1_gamma)
    cpar_np[:, 7] = 1.0 / dup(bn2_gamma)

    in_maps = []
    for k in range(NCORES):
        xmain_np = np.zeros((NPAIR, 128, PP), np.float16)
        xcross_np = np.zeros((NPAIR, 128, 2, PP), E4)
        for p in range(NPAIR):
            tt_, bp = p // 2, p % 2
            b0 = 4 * k + bp * 2
            # main: [xhA; xhB]
            mm = np.zeros((128, HP, HP), np.float32)
            mm[0:64, 1:57, 1:57] = xh_t[tt_, b0]
            mm[64:128, 1:57, 1:57] = xh_t[tt_, b0 + 1]
            xmain_np[p] = mm.reshape(128, PP).astype(np.float16)
            # cross planes: per image [512*xl ; xh/8]
            for j in range(2):
                cp = np.zeros((128, HP, HP), np.float32)
                cp[0:64, 1:57, 1:57] = 512.0 * xl_t[tt_, b0 + j]
                cp[64:128, 1:57, 1:57] = xh_t[tt_, b0 + j] / 8.0
                xcross_np[p, :, j, :] = cp.reshape(128, PP).astype(E4)
        xin_np = np.ascontiguousarray(
            x[:, 4 * k:4 * k + 4].reshape(NIMG, 64, PIX))
        in_maps.append({
            names['xmain']: xmain_np,
            names['xcross']: xcross_np,
            names['xin']: xin_np,
            names['w1m']: w1m_np,
            names['w1x']: w1x_np,
            names['w2a']: w2a_np,
            names['w2b']: w2b_np,
            names['cpar']: cpar_np,
        })

    return nc, names, in_maps


def kernel(**inputs):
    from concourse.bass_utils import run_bass_kernel_spmd
    nc, names, in_maps = prepare(**inputs)
    res = run_bass_kernel_spmd(nc, in_maps, core_ids=list(range(NCORES)))
    global LAST_RES, LAST_NAMES
    LAST_RES, LAST_NAMES = res, names
    out = np.empty((T, B, C, H, W), np.float32)
    for k in range(NCORES):
        o = res.results[k][names['outp']]
        out[:, 4 * k:4 * k + 4] = o.reshape(T, BL, C, H, W)
    return out


if __name__ == "__main__":
    rng = np.random.default_rng(0)
    xs = rng.standard_normal((T, B, C, H, W)).astype(np.float32)
    w1 = (rng.standard_normal((64, 64, 3, 3)) * 0.05).astype(np.float32)
    w2 = (rng.standard_normal((64, 64, 3, 3)) * 0.05).astype(np.float32)
    o = kernel(xs, w1, np.ones(64, np.float32), np.zeros(64, np.float32),
               np.zeros(1, np.float32), w2, np.ones(64, np.float32),
               np.zeros(64, np.float32), np.zeros(1, np.float32))
    print("ran:", o.shape, float(o.mean()))
